# revision 10
# baseline (speedup 1.0000x reference)
"""Trainium2 Bass kernel for nn_FFF (fast-feedforward tree routing).

Strategy (data-parallel over 8 cores, batch-sharded):
  Per core (B_c = 8192 samples), dense formulation:
    1. L[b, e] = x[b] . w1[e]  for ALL 1023 nodes (PE, fp32r + bf16 correction
       terms for the routing nodes so branch signs match fp32).
    2. Path mask via level-by-level recurrence on DVE (node axis on the free
       dim, level-major storage so all ops are contiguous):
         m[right child] = m[parent] * (L[parent] > 0);  m[left] = m - right.
    3. C = m * L  (10 nonzeros per row), Y = C @ W2 on PE (fp32r), with C
       transposed 128x128 block-wise on the PE so the contraction runs over
       nodes.
  Node storage permutation: level d occupies positions [2^d, 2^{d+1});
  within a level, children of level-d parents are stored [left-block |
  right-block].  The weight tables are permuted on the host to match, which
  makes every mask op a contiguous slice.  Position 0 is an all-zero dummy.

  fp32r rounds operands to 11 mantissa bits; routing signs need better, so
  lam_routing = f32r(x).f32r(w) + bf16(x_lo).bf16(w) + bf16(x).bf16(w_lo),
  which matches fp32 signs for all practical purposes (verified ~1e-4 final
  rel err vs the fp32 reference, no routing flips at B=65536).
"""

import os
import numpy as np
import ml_dtypes
from contextlib import ExitStack

import concourse.bass as bass  # noqa: F401  (AP helpers)
import concourse.tile as tile
from concourse import bacc, mybir
from concourse.bass_utils import run_bass_kernel_spmd
from concourse.masks import make_identity

F32 = mybir.dt.float32
F32R = mybir.dt.float32r
BF16 = mybir.dt.bfloat16

N_CORES = 8
B_FULL, NIN, NOUT = 65536, 1024, 1024
BC = B_FULL // N_CORES          # 8192 samples per core
DEPTH = 10
NN = 1024                        # node positions (0 = dummy, 1..1023 = nodes)
TB = 256                         # sample tile (2 blocks of 128)
NBLK = 128                       # PE output block (samples)

_CACHE = {}


def _build_nc():
    if "nc" in _CACHE:
        return _CACHE["nc"]
    nc = bacc.Bacc("TRN2", target_bir_lowering=False, debug=False,
                   enable_asserts=False, num_devices=N_CORES)

    xt_d = nc.dram_tensor("xt", [NIN, BC], F32R, kind="ExternalInput").ap()
    xbf_d = nc.dram_tensor("xbf", [NIN, BC], BF16, kind="ExternalInput").ap()
    xlo_d = nc.dram_tensor("xlo", [NIN, BC], BF16, kind="ExternalInput").ap()
    w1t_d = nc.dram_tensor("w1t", [NIN, NN], F32R, kind="ExternalInput").ap()
    w1tbf_d = nc.dram_tensor("w1tbf", [NIN, 128], BF16, kind="ExternalInput").ap()
    w1tlo_d = nc.dram_tensor("w1tlo", [NIN, 128], BF16, kind="ExternalInput").ap()
    w2_d = nc.dram_tensor("w2", [NN, NOUT], BF16, kind="ExternalInput").ap()
    iotab_d = nc.dram_tensor("iotab", [128, 512], F32, kind="ExternalInput").ap()
    y_d = nc.dram_tensor("y", [BC, NOUT], F32, kind="ExternalOutput").ap()

    with tile.TileContext(nc) as tc:
        with ExitStack() as ctx:
            statics = ctx.enter_context(tc.tile_pool(name="statics", bufs=1))
            xpool = ctx.enter_context(tc.tile_pool(name="xpool", bufs=4))
            lpool = ctx.enter_context(tc.tile_pool(name="lpool", bufs=4))
            mpool = ctx.enter_context(tc.tile_pool(name="mpool", bufs=4))
            cpool = ctx.enter_context(tc.tile_pool(name="cpool", bufs=4))
            ctpool = ctx.enter_context(tc.tile_pool(name="ctpool", bufs=6))
            ypool = ctx.enter_context(tc.tile_pool(name="ypool", bufs=4))
            psumL = ctx.enter_context(tc.tile_pool(name="psumL", bufs=2, space="PSUM"))
            psumT = ctx.enter_context(tc.tile_pool(name="psumT", bufs=2, space="PSUM"))
            psumY = ctx.enter_context(tc.tile_pool(name="psumY", bufs=2, space="PSUM"))

            ident = statics.tile([128, 128], F32)
            make_identity(nc, ident[:])
            identb = statics.tile([128, 128], BF16)
            nc.vector.tensor_copy(identb[:], ident[:])

            w1t_sb = statics.tile([128, 8, NN], F32R)
            nc.sync.dma_start(w1t_sb[:], w1t_d.rearrange("(ic p) e -> p ic e", p=128))
            w1tbf_sb = statics.tile([128, 8, 128], BF16)
            nc.sync.dma_start(w1tbf_sb[:], w1tbf_d.rearrange("(ic p) e -> p ic e", p=128))
            w1tlo_sb = statics.tile([128, 8, 128], BF16)
            nc.sync.dma_start(w1tlo_sb[:], w1tlo_d.rearrange("(ic p) e -> p ic e", p=128))
            w2_sb = statics.tile([128, 4, NOUT], BF16)
            nc.sync.dma_start(w2_sb[:], w2_d[0:512].rearrange("(ec p) n -> p ec n", p=128))
            iota_sb = statics.tile([128, 512], F32)
            nc.sync.dma_start(iota_sb[:], iotab_d[:])

            xt_r = xt_d.rearrange("(ic p) b -> p ic b", p=128)
            xbf_r = xbf_d.rearrange("(ic p) b -> p ic b", p=128)
            xlo_r = xlo_d.rearrange("(ic p) b -> p ic b", p=128)

            n_tiles = BC // TB
            blocks_per_tile = TB // NBLK
            for t in range(n_tiles):
                bsl = slice(t * TB, (t + 1) * TB)
                xt_sb = xpool.tile([128, 8, TB], F32R, tag="xt")
                nc.sync.dma_start(xt_sb[:], xt_r[:, :, bsl])
                xbf_sb = xpool.tile([128, 8, TB], BF16, tag="xbf")
                nc.sync.dma_start(xbf_sb[:], xbf_r[:, :, bsl])
                xlo_sb = xpool.tile([128, 8, TB], BF16, tag="xlo")
                nc.sync.dma_start(xlo_sb[:], xlo_r[:, :, bsl])

                L_sb = lpool.tile([128, blocks_per_tile, NN], BF16)
                # ---- mm A ----
                for jb in range(blocks_per_tile):
                    jsl = slice(jb * NBLK, (jb + 1) * NBLK)
                    plr = psumL.tile([128, 512], F32, tag="plr")
                    for ic in range(8):
                        nc.tensor.matmul(
                            plr[:],
                            lhsT=xt_sb[:, ic, jsl],
                            rhs=w1t_sb[:, ic, 0:512],
                            start=(ic == 0), stop=False, skip_group_check=True,
                        )
                    for ic in range(8):
                        nc.tensor.matmul(
                            plr[:, 0:128],
                            lhsT=xlo_sb[:, ic, jsl],
                            rhs=w1tbf_sb[:, ic, :],
                            start=False, stop=False,
                        )
                    for ic in range(8):
                        nc.tensor.matmul(
                            plr[:, 0:128],
                            lhsT=xbf_sb[:, ic, jsl],
                            rhs=w1tlo_sb[:, ic, :],
                            start=False, stop=(ic == 7),
                        )
                    nc.any.tensor_copy(L_sb[:, jb, 0:512], plr[:])
                    # leaf half (positions 512..1023): 1 term
                    pll = psumL.tile([128, 512], F32, tag="pll")
                    for ic in range(8):
                        nc.tensor.matmul(
                            pll[:],
                            lhsT=xt_sb[:, ic, jsl],
                            rhs=w1t_sb[:, ic, 512:1024],
                            start=(ic == 0), stop=(ic == 7),
                        )
                    nc.any.tensor_copy(L_sb[:, jb, 512:1024], pll[:])

                # ---- routing masks (1-D ops per block for DVE 2x mode) ----
                gt_sb = mpool.tile([128, blocks_per_tile, 512], BF16, tag="gt")
                m_sb = mpool.tile([128, blocks_per_tile, NN], BF16, tag="m")
                for jb in range(blocks_per_tile):
                    nc.vector.tensor_single_scalar(
                        gt_sb[:, jb, :], L_sb[:, jb, 0:512], 0.0,
                        mybir.AluOpType.is_gt)
                    nc.vector.memset(m_sb[:, jb, 0:2], 0.0)
                    nc.vector.memset(m_sb[:, jb, 1:2], 1.0)
                    for d in range(DEPTH - 1):
                        s = 2 ** d   # level-d block [s, 2s)
                        n = 2 ** d
                        nc.vector.tensor_mul(
                            m_sb[:, jb, 2 * s + n: 2 * s + 2 * n],
                            m_sb[:, jb, s: s + n], gt_sb[:, jb, s: s + n])
                        nc.vector.tensor_sub(
                            m_sb[:, jb, 2 * s: 2 * s + n],
                            m_sb[:, jb, s: s + n],
                            m_sb[:, jb, 2 * s + n: 2 * s + 2 * n])

                # ---- leaf (level 9): per-sample position and lam ----
                scr = mpool.tile([128, 512], F32, tag="scr")
                scrb = mpool.tile([128, 512], BF16, tag="scrb")
                pos9f = mpool.tile([128, blocks_per_tile, 1], F32, tag="pos9f")
                lam9 = mpool.tile([128, blocks_per_tile, 1], F32, tag="lam9")
                pos9i = mpool.tile([128, blocks_per_tile, 1], mybir.dt.int32, tag="pos9i")
                for jb in range(blocks_per_tile):
                    nc.vector.tensor_mul(scr[:], m_sb[:, jb, 512:1024], iota_sb[:])
                    nc.vector.reduce_sum(pos9f[:, jb, :], scr[:],
                                         axis=mybir.AxisListType.X)
                    nc.vector.tensor_mul(scrb[:], m_sb[:, jb, 512:1024],
                                         L_sb[:, jb, 512:1024])
                    nc.vector.reduce_sum(lam9[:, jb, :], scrb[:],
                                         axis=mybir.AxisListType.X)
                nc.vector.tensor_copy(pos9i[:], pos9f[:])

                # ---- C = m * L (routing positions only) ----
                C_sb = cpool.tile([128, blocks_per_tile, 512], BF16)
                for jb in range(blocks_per_tile):
                    nc.vector.tensor_mul(C_sb[:, jb, :], m_sb[:, jb, 0:512],
                                         L_sb[:, jb, 0:512])

                # ---- transpose C and mm B, per block ----
                for jb in range(blocks_per_tile):
                    ct_sb = ctpool.tile([128, 4, 128], BF16, tag="ct")
                    pt = psumT.tile([128, 512], BF16)
                    for k in range(4):
                        nc.tensor.transpose(
                            pt[:, k * 128:(k + 1) * 128],
                            C_sb[:, jb, k * 128:(k + 1) * 128], identb[:])
                    nc.any.tensor_copy(
                        ct_sb[:].rearrange("p a b -> p (a b)"), pt[:])

                    w2g = ctpool.tile([128, NOUT], BF16, tag="w2g")
                    nc.gpsimd.indirect_dma_start(
                        out=w2g[:], out_offset=None, in_=w2_d[:],
                        in_offset=bass.IndirectOffsetOnAxis(
                            ap=pos9i[:, jb, :], axis=0))

                    y_sb = ypool.tile([128, NOUT], F32)
                    for nh in range(2):
                        py = psumY.tile([128, 512], F32)
                        for ec in range(4):
                            nc.tensor.matmul(
                                py[:],
                                lhsT=ct_sb[:, ec, :],
                                rhs=w2_sb[:, ec, nh * 512:(nh + 1) * 512],
                                start=(ec == 0), stop=(ec == 3),
                            )
                        nc.any.tensor_copy(y_sb[:, nh * 512:(nh + 1) * 512], py[:])
                    nc.vector.scalar_tensor_tensor(
                        out=y_sb[:], in0=w2g[:], scalar=lam9[:, jb, :], in1=y_sb[:],
                        op0=mybir.AluOpType.mult, op1=mybir.AluOpType.add)
                    nc.sync.dma_start(y_d[t * TB + jb * NBLK: t * TB + (jb + 1) * NBLK, :],
                                      y_sb[:])

    nc.compile()
    _CACHE["nc"] = nc
    return nc


def _build_perm():
    """perm[pos-1] = original node id for storage position pos (1..1023)."""
    perm = [0]
    nodes = [0]
    for _ in range(DEPTH - 1):
        nxt = [2 * v + 1 for v in nodes] + [2 * v + 2 for v in nodes]
        perm += nxt
        nodes = nxt
    return np.array(perm, dtype=np.int64)


def _rne11(x):
    """Round-to-nearest-even at 11 mantissa bits (fp32r's operand rounding)."""
    xi = x.view(np.uint32).astype(np.uint64)
    shift = np.uint64(12)
    lsb_mask = np.uint64((1 << 12) - 1)
    half = np.uint64(1 << 11)
    frac = xi & lsb_mask
    base = xi >> shift
    roundup = (frac > half) | ((frac == half) & ((base & np.uint64(1)) == 1))
    out = (base + roundup.astype(np.uint64)) << shift
    return out.astype(np.uint32).view(np.float32)


def kernel(x, w1s, w2s):
    nc = _build_nc()

    perm = _build_perm()
    w1p = np.ascontiguousarray(w1s[perm])          # [1023, 1024]
    w2p = np.ascontiguousarray(w2s[perm])

    w1t = np.zeros((NIN, NN), dtype=np.float32)    # [i, pos]
    w1t[:, 1:] = w1p.T
    w2f = np.zeros((NN, NOUT), dtype=np.float32)
    w2f[1:] = w2p
    w2bf = w2f.astype(ml_dtypes.bfloat16)
    iotab = np.tile(np.arange(512, 1024, dtype=np.float32), (128, 1))

    w1t_route = w1t[:, 0:128]
    w1tbf = w1t_route.astype(ml_dtypes.bfloat16)
    w1tlo = (w1t_route - _rne11(w1t_route)).astype(ml_dtypes.bfloat16)

    xt = np.ascontiguousarray(x.T)                 # [1024, 65536]
    xbf = xt.astype(ml_dtypes.bfloat16)
    xlo = (xt - _rne11(xt)).astype(ml_dtypes.bfloat16)

    in_maps = []
    for c in range(N_CORES):
        csl = slice(c * BC, (c + 1) * BC)
        in_maps.append({
            "xt": np.ascontiguousarray(xt[:, csl]),
            "xbf": np.ascontiguousarray(xbf[:, csl]),
            "xlo": np.ascontiguousarray(xlo[:, csl]),
            "w1t": w1t, "w1tbf": w1tbf, "w1tlo": w1tlo, "w2": w2bf,
            "iotab": iotab,
        })

    trace = bool(int(os.environ.get("FFF_TRACE", "0")))
    res = run_bass_kernel_spmd(nc, in_maps, core_ids=list(range(N_CORES)),
                               trace=trace)
    _CACHE["last_result"] = res
    y = np.concatenate([res.results[c]["y"] for c in range(N_CORES)], axis=0)
    return y


# revision 11
# speedup vs baseline: 1.1192x; 1.1192x over previous
"""Trainium2 Bass kernel for nn_FFF (fast-feedforward tree routing).

Strategy (data-parallel over 8 cores, batch-sharded):
  Per core (B_c = 8192 samples), dense formulation:
    1. L[b, e] = x[b] . w1[e]  for ALL 1023 nodes (PE, fp32r + bf16 correction
       terms for the routing nodes so branch signs match fp32).
    2. Path mask via level-by-level recurrence on DVE (node axis on the free
       dim, level-major storage so all ops are contiguous):
         m[right child] = m[parent] * (L[parent] > 0);  m[left] = m - right.
    3. C = m * L  (10 nonzeros per row), Y = C @ W2 on PE (fp32r), with C
       transposed 128x128 block-wise on the PE so the contraction runs over
       nodes.
  Node storage permutation: level d occupies positions [2^d, 2^{d+1});
  within a level, children of level-d parents are stored [left-block |
  right-block].  The weight tables are permuted on the host to match, which
  makes every mask op a contiguous slice.  Position 0 is an all-zero dummy.

  fp32r rounds operands to 11 mantissa bits; routing signs need better, so
  lam_routing = f32r(x).f32r(w) + bf16(x_lo).bf16(w) + bf16(x).bf16(w_lo),
  which matches fp32 signs for all practical purposes (verified ~1e-4 final
  rel err vs the fp32 reference, no routing flips at B=65536).
"""

import os
import numpy as np
import ml_dtypes
from contextlib import ExitStack

import concourse.bass as bass  # noqa: F401  (AP helpers)
import concourse.tile as tile
from concourse import bacc, mybir
from concourse.bass_utils import run_bass_kernel_spmd
from concourse.masks import make_identity

F32 = mybir.dt.float32
F32R = mybir.dt.float32r
BF16 = mybir.dt.bfloat16

N_CORES = 8
B_FULL, NIN, NOUT = 65536, 1024, 1024
BC = B_FULL // N_CORES          # 8192 samples per core
DEPTH = 10
NN = 1024                        # node positions (0 = dummy, 1..1023 = nodes)
TB = 256                         # sample tile (2 blocks of 128)
NBLK = 128                       # PE output block (samples)

_CACHE = {}


def _build_nc():
    if "nc" in _CACHE:
        return _CACHE["nc"]
    nc = bacc.Bacc("TRN2", target_bir_lowering=False, debug=False,
                   enable_asserts=False, num_devices=N_CORES)

    xt_d = nc.dram_tensor("xt", [NIN, BC], F32R, kind="ExternalInput").ap()
    xbf_d = nc.dram_tensor("xbf", [NIN, BC], BF16, kind="ExternalInput").ap()
    xlo_d = nc.dram_tensor("xlo", [NIN, BC], BF16, kind="ExternalInput").ap()
    w1t_d = nc.dram_tensor("w1t", [NIN, NN], F32R, kind="ExternalInput").ap()
    w1tbf_d = nc.dram_tensor("w1tbf", [NIN, 128], BF16, kind="ExternalInput").ap()
    w1tlo_d = nc.dram_tensor("w1tlo", [NIN, 128], BF16, kind="ExternalInput").ap()
    w2_d = nc.dram_tensor("w2", [NN, NOUT], BF16, kind="ExternalInput").ap()
    iotab_d = nc.dram_tensor("iotab", [128, 512], F32, kind="ExternalInput").ap()
    y_d = nc.dram_tensor("y", [BC, NOUT], F32, kind="ExternalOutput").ap()

    with tile.TileContext(nc) as tc:
        with ExitStack() as ctx:
            statics = ctx.enter_context(tc.tile_pool(name="statics", bufs=1))
            xpool = ctx.enter_context(tc.tile_pool(name="xpool", bufs=3))
            lpool = ctx.enter_context(tc.tile_pool(name="lpool", bufs=3))
            mpool = ctx.enter_context(tc.tile_pool(name="mpool", bufs=3))
            cpool = ctx.enter_context(tc.tile_pool(name="cpool", bufs=3))
            ctpool = ctx.enter_context(tc.tile_pool(name="ctpool", bufs=4))
            ypool = ctx.enter_context(tc.tile_pool(name="ypool", bufs=3))
            psumL = ctx.enter_context(tc.tile_pool(name="psumL", bufs=2, space="PSUM"))
            psumT = ctx.enter_context(tc.tile_pool(name="psumT", bufs=2, space="PSUM"))
            psumY = ctx.enter_context(tc.tile_pool(name="psumY", bufs=2, space="PSUM"))

            ident = statics.tile([128, 128], F32)
            make_identity(nc, ident[:])
            identb = statics.tile([128, 128], BF16)
            nc.vector.tensor_copy(identb[:], ident[:])

            w1t_sb = statics.tile([128, 8, NN], F32R)
            nc.sync.dma_start(w1t_sb[:], w1t_d.rearrange("(ic p) e -> p ic e", p=128))
            w1tbf_sb = statics.tile([128, 8, 128], BF16)
            nc.sync.dma_start(w1tbf_sb[:], w1tbf_d.rearrange("(ic p) e -> p ic e", p=128))
            w1tlo_sb = statics.tile([128, 8, 128], BF16)
            nc.sync.dma_start(w1tlo_sb[:], w1tlo_d.rearrange("(ic p) e -> p ic e", p=128))
            w2_sb = statics.tile([128, 4, NOUT], BF16)
            nc.sync.dma_start(w2_sb[:], w2_d[0:512].rearrange("(ec p) n -> p ec n", p=128))
            iota_sb = statics.tile([128, 512], F32)
            nc.sync.dma_start(iota_sb[:], iotab_d[:])

            xt_r = xt_d.rearrange("(ic p) b -> p ic b", p=128)
            xbf_r = xbf_d.rearrange("(ic p) b -> p ic b", p=128)
            xlo_r = xlo_d.rearrange("(ic p) b -> p ic b", p=128)

            n_tiles = BC // TB
            blocks_per_tile = TB // NBLK
            for t in range(n_tiles):
                bsl = slice(t * TB, (t + 1) * TB)
                xt_sb = xpool.tile([128, 8, TB], F32R, tag="xt")
                nc.sync.dma_start(xt_sb[:], xt_r[:, :, bsl])
                xbf_sb = xpool.tile([128, 8, TB], BF16, tag="xbf")
                nc.sync.dma_start(xbf_sb[:], xbf_r[:, :, bsl])
                xlo_sb = xpool.tile([128, 8, TB], BF16, tag="xlo")
                nc.sync.dma_start(xlo_sb[:], xlo_r[:, :, bsl])

                L_sb = lpool.tile([128, blocks_per_tile, NN], F32)
                # ---- mm A ----
                for jb in range(blocks_per_tile):
                    jsl = slice(jb * NBLK, (jb + 1) * NBLK)
                    plr = psumL.tile([128, 512], F32, tag="plr")
                    for ic in range(8):
                        nc.tensor.matmul(
                            plr[:],
                            lhsT=xt_sb[:, ic, jsl],
                            rhs=w1t_sb[:, ic, 0:512],
                            start=(ic == 0), stop=False, skip_group_check=True,
                        )
                    for ic in range(8):
                        nc.tensor.matmul(
                            plr[:, 0:128],
                            lhsT=xlo_sb[:, ic, jsl],
                            rhs=w1tbf_sb[:, ic, :],
                            start=False, stop=False,
                        )
                    for ic in range(8):
                        nc.tensor.matmul(
                            plr[:, 0:128],
                            lhsT=xbf_sb[:, ic, jsl],
                            rhs=w1tlo_sb[:, ic, :],
                            start=False, stop=(ic == 7),
                        )
                    nc.any.tensor_copy(L_sb[:, jb, 0:512], plr[:])
                    # leaf half (positions 512..1023): 1 term
                    pll = psumL.tile([128, 512], F32, tag="pll")
                    for ic in range(8):
                        nc.tensor.matmul(
                            pll[:],
                            lhsT=xt_sb[:, ic, jsl],
                            rhs=w1t_sb[:, ic, 512:1024],
                            start=(ic == 0), stop=(ic == 7),
                        )
                    nc.any.tensor_copy(L_sb[:, jb, 512:1024], pll[:])

                # ---- routing masks (1-D ops per block for DVE 2x mode) ----
                gt_sb = mpool.tile([128, blocks_per_tile, 512], BF16, tag="gt")
                m_sb = mpool.tile([128, blocks_per_tile, NN], BF16, tag="m")
                for jb in range(blocks_per_tile):
                    nc.vector.tensor_single_scalar(
                        gt_sb[:, jb, :], L_sb[:, jb, 0:512], 0.0,
                        mybir.AluOpType.is_gt)
                    nc.vector.memset(m_sb[:, jb, 0:2], 0.0)
                    nc.vector.memset(m_sb[:, jb, 1:2], 1.0)
                    for d in range(DEPTH - 1):
                        s = 2 ** d   # level-d block [s, 2s)
                        n = 2 ** d
                        nc.vector.tensor_mul(
                            m_sb[:, jb, 2 * s + n: 2 * s + 2 * n],
                            m_sb[:, jb, s: s + n], gt_sb[:, jb, s: s + n])
                        nc.vector.tensor_sub(
                            m_sb[:, jb, 2 * s: 2 * s + n],
                            m_sb[:, jb, s: s + n],
                            m_sb[:, jb, 2 * s + n: 2 * s + 2 * n])

                # ---- leaf (level 9): per-sample position and lam ----
                scr = mpool.tile([128, 512], F32, tag="scr")
                pos9f = mpool.tile([128, blocks_per_tile, 1], F32, tag="pos9f")
                lam9 = mpool.tile([128, blocks_per_tile, 1], F32, tag="lam9")
                pos9i = mpool.tile([128, blocks_per_tile, 1], mybir.dt.int32, tag="pos9i")
                for jb in range(blocks_per_tile):
                    nc.vector.tensor_mul(scr[:], m_sb[:, jb, 512:1024], iota_sb[:])
                    nc.vector.reduce_sum(pos9f[:, jb, :], scr[:],
                                         axis=mybir.AxisListType.X)
                    nc.vector.tensor_mul(scr[:], m_sb[:, jb, 512:1024],
                                         L_sb[:, jb, 512:1024])
                    nc.vector.reduce_sum(lam9[:, jb, :], scr[:],
                                         axis=mybir.AxisListType.X)
                nc.vector.tensor_copy(pos9i[:], pos9f[:])

                # ---- C = m * L (routing positions only) ----
                C_sb = cpool.tile([128, blocks_per_tile, 512], BF16)
                nc.vector.tensor_mul(C_sb[:], m_sb[:, :, 0:512], L_sb[:, :, 0:512])

                # ---- transpose C and mm B, per block ----
                for jb in range(blocks_per_tile):
                    ct_sb = ctpool.tile([128, 4, 128], BF16, tag="ct")
                    pt = psumT.tile([128, 512], BF16)
                    for k in range(4):
                        nc.tensor.transpose(
                            pt[:, k * 128:(k + 1) * 128],
                            C_sb[:, jb, k * 128:(k + 1) * 128], identb[:])
                    nc.any.tensor_copy(
                        ct_sb[:].rearrange("p a b -> p (a b)"), pt[:])

                    w2g = ctpool.tile([128, NOUT], BF16, tag="w2g")
                    nc.gpsimd.indirect_dma_start(
                        out=w2g[:], out_offset=None, in_=w2_d[:],
                        in_offset=bass.IndirectOffsetOnAxis(
                            ap=pos9i[:, jb, :], axis=0))

                    y_sb = ypool.tile([128, NOUT], F32)
                    for nh in range(2):
                        py = psumY.tile([128, 512], F32)
                        for ec in range(4):
                            nc.tensor.matmul(
                                py[:],
                                lhsT=ct_sb[:, ec, :],
                                rhs=w2_sb[:, ec, nh * 512:(nh + 1) * 512],
                                start=(ec == 0), stop=(ec == 3),
                            )
                        nc.any.tensor_copy(y_sb[:, nh * 512:(nh + 1) * 512], py[:])
                    nc.vector.scalar_tensor_tensor(
                        out=y_sb[:], in0=w2g[:], scalar=lam9[:, jb, :], in1=y_sb[:],
                        op0=mybir.AluOpType.mult, op1=mybir.AluOpType.add)
                    nc.sync.dma_start(y_d[t * TB + jb * NBLK: t * TB + (jb + 1) * NBLK, :],
                                      y_sb[:])

    nc.compile()
    _CACHE["nc"] = nc
    return nc


def _build_perm():
    """perm[pos-1] = original node id for storage position pos (1..1023)."""
    perm = [0]
    nodes = [0]
    for _ in range(DEPTH - 1):
        nxt = [2 * v + 1 for v in nodes] + [2 * v + 2 for v in nodes]
        perm += nxt
        nodes = nxt
    return np.array(perm, dtype=np.int64)


def _rne11(x):
    """Round-to-nearest-even at 11 mantissa bits (fp32r's operand rounding)."""
    xi = x.view(np.uint32).astype(np.uint64)
    shift = np.uint64(12)
    lsb_mask = np.uint64((1 << 12) - 1)
    half = np.uint64(1 << 11)
    frac = xi & lsb_mask
    base = xi >> shift
    roundup = (frac > half) | ((frac == half) & ((base & np.uint64(1)) == 1))
    out = (base + roundup.astype(np.uint64)) << shift
    return out.astype(np.uint32).view(np.float32)


def kernel(x, w1s, w2s):
    nc = _build_nc()

    perm = _build_perm()
    w1p = np.ascontiguousarray(w1s[perm])          # [1023, 1024]
    w2p = np.ascontiguousarray(w2s[perm])

    w1t = np.zeros((NIN, NN), dtype=np.float32)    # [i, pos]
    w1t[:, 1:] = w1p.T
    w2f = np.zeros((NN, NOUT), dtype=np.float32)
    w2f[1:] = w2p
    w2bf = w2f.astype(ml_dtypes.bfloat16)
    iotab = np.tile(np.arange(512, 1024, dtype=np.float32), (128, 1))

    w1t_route = w1t[:, 0:128]
    w1tbf = w1t_route.astype(ml_dtypes.bfloat16)
    w1tlo = (w1t_route - _rne11(w1t_route)).astype(ml_dtypes.bfloat16)

    xt = np.ascontiguousarray(x.T)                 # [1024, 65536]
    xbf = xt.astype(ml_dtypes.bfloat16)
    xlo = (xt - _rne11(xt)).astype(ml_dtypes.bfloat16)

    in_maps = []
    for c in range(N_CORES):
        csl = slice(c * BC, (c + 1) * BC)
        in_maps.append({
            "xt": np.ascontiguousarray(xt[:, csl]),
            "xbf": np.ascontiguousarray(xbf[:, csl]),
            "xlo": np.ascontiguousarray(xlo[:, csl]),
            "w1t": w1t, "w1tbf": w1tbf, "w1tlo": w1tlo, "w2": w2bf,
            "iotab": iotab,
        })

    trace = bool(int(os.environ.get("FFF_TRACE", "0")))
    res = run_bass_kernel_spmd(nc, in_maps, core_ids=list(range(N_CORES)),
                               trace=trace)
    _CACHE["last_result"] = res
    y = np.concatenate([res.results[c]["y"] for c in range(N_CORES)], axis=0)
    return y
